# Initial kernel scaffold
#
"""Trainium2 Bass kernel for nn_CapsuleLayer (EM-routing capsule layer).

Strategy
--------
Shard the 196 (h,w) pixel positions across 8 cores (25/25/25/25/24/24/24/24,
padded to 25 slots each).  Each (h,w) "tile" covers the 16 batch elements
(16 "pixels" per tile).

On-chip layout: partition q = (n, i)  [n = out-capsule 0..31, i = pose-row
0..3], free = (pix, m, k) [pix = batch 0..15, m = in-capsule 0..31, k =
pose-col 0..3].  d = (i, k).

votes[q, (pix,m,k)] are produced by one PE matmul per tile:
  lhsT = W[hw] as [(m,j), (n,i)]  (128x128)
  rhs  = block-diag pose [(m,j), (pix,m',k)] (built on host, 128x2048)
  out[(n,i),(pix,m,k)] = sum_j W[m,n,i,j] * pose[pix,m,j,k]

Routing (3 EM iterations) runs with:
  - DVE: elementwise tensor-tensor passes + small ops/reciprocals
  - ACT: squares / exp / ln / sigmoid
  - GPSIMD: two of the big elementwise passes + cross-partition max
  - PE: all reductions over m (32 accumulating identity matmuls), over
    (i,k) (block-diag matmuls), over n (all-ones matmuls)

sum over m uses unnormalized priors p; the 1/sum_m normalization is applied
to the (tiny) reduced outputs instead of the big tensors.
softmax over n: exact max via gpsimd.partition_all_reduce; the 1/sum_n
normalization is folded through ln/exp (values are positive there).
"""

import os
import sys

import numpy as np

sys.path.insert(0, "/opt/trn_rl_repo")

import concourse.bass as bass
import concourse.tile as tile
from concourse import mybir
from concourse.bass_utils import run_bass_kernel_spmd
from bass_rust import bass_isa

F32 = mybir.dt.float32
BF16 = mybir.dt.bfloat16
AF = mybir.ActivationFunctionType
OP = mybir.AluOpType
AX = mybir.AxisListType

EPS = 1e-7
TWO_PI = 2.0 * np.pi
LNC32 = float(np.log(1.0 / 32.0 + EPS))
LN4 = float(np.log(4.0))

N_CORES = 8
HW = 196
SLOTS = 25  # padded hw tiles per core
PIX = 16  # batch elements per hw tile
COUNTS = [25, 25, 25, 25, 24, 24, 24, 24]
OFFS = np.cumsum([0] + COUNTS)[:-1]

# dtype of the PE-feeding product tensors (PV/PS/T2).  bf16 makes those
# matmul rhs operands run at 1 cycle/row instead of 4.
PE_RHS_BF16 = True

_CACHED = {}


def _bc(ap, dims):
    """View `ap` (a [128, ...] SBUF/PSUM AP) with custom free dims."""
    return bass.AP(tensor=ap.tensor, offset=ap.offset, ap=[ap.ap[0]] + dims)


def build_bass():
    nc = bass.Bass()
    wl_d = nc.dram_tensor("wl", [SLOTS, 128, 128], F32, kind="ExternalInput")
    rhs_d = nc.dram_tensor("rhs", [SLOTS, 128, 2048], F32, kind="ExternalInput")
    cp_d = nc.dram_tensor("cp", [SLOTS, 128, 512], F32, kind="ExternalInput")
    cst_d = nc.dram_tensor("cst", [4, 128, 128], F32, kind="ExternalInput")
    out_d = nc.dram_tensor("out", [SLOTS, PIX, 544], F32, kind="ExternalOutput")

    prhs_dt = BF16 if PE_RHS_BF16 else F32

    with tile.TileContext(nc) as tc:
        with (
            tc.tile_pool(name="consts", bufs=1) as consts,
            tc.tile_pool(name="win", bufs=2) as win,
            tc.tile_pool(name="rin", bufs=2) as rin,
            tc.tile_pool(name="cin", bufs=2) as cin,
            tc.tile_pool(name="vbuf", bufs=2) as vbuf,
            tc.tile_pool(name="big", bufs=3) as big,
            tc.tile_pool(name="sq", bufs=2) as sqp,
            tc.tile_pool(name="med", bufs=3) as med,
            tc.tile_pool(name="sml", bufs=4) as sml,
            tc.tile_pool(name="tin", bufs=6) as tin,
            tc.tile_pool(name="ops", bufs=2) as ops,
            tc.tile_pool(name="ps_v", bufs=1, space="PSUM") as ps_v,
            tc.tile_pool(name="ps_mu", bufs=2, space="PSUM") as ps_mu,
            tc.tile_pool(name="ps_sq", bufs=1, space="PSUM") as ps_sq,
            tc.tile_pool(name="ps_d", bufs=1, space="PSUM") as ps_d,
        ):
            ident = consts.tile([128, 128], F32, tag="ident")
            bdiag = consts.tile([128, 128], F32, tag="bdiag")
            onesm = consts.tile([128, 128], F32, tag="onesm")
            bias16 = consts.tile([128, 1], F32, tag="bias16")
            nc.sync.dma_start(out=ident, in_=cst_d[0])
            nc.sync.dma_start(out=bdiag, in_=cst_d[1])
            nc.sync.dma_start(out=onesm, in_=cst_d[2])
            nc.sync.dma_start(out=bias16, in_=cst_d[3, :, 0:1])

            for t in range(SLOTS):
                wl_t = win.tile([128, 128], F32, tag="wl")
                rhs_t = rin.tile([128, 2048], F32, tag="rhs")
                cp_t = cin.tile([128, 512], F32, tag="cp")
                nc.sync.dma_start(out=wl_t, in_=wl_d[t])
                nc.sync.dma_start(out=rhs_t, in_=rhs_d[t])
                nc.sync.dma_start(out=cp_t, in_=cp_d[t])

                votes_ps = ps_v.tile([128, 2048], F32, tag="votes")
                nc.tensor.matmul(votes_ps, wl_t, rhs_t, start=True, stop=True)
                V = vbuf.tile([128, 2048], F32, tag="V")
                nc.vector.tensor_copy(V[:, 0:1024], votes_ps[:, 0:1024])
                nc.scalar.copy(V[:, 1024:2048], votes_ps[:, 1024:2048])
                V4 = V[:].rearrange("q (pix m k) -> q pix m k", pix=PIX, m=32)

                p = cp_t  # iter-0 priors_in == normalized coeffs
                rs = None  # 1/sum_m p (None => sum is exactly 1)
                spo_save = None
                mu_sb = None

                for it in range(3):
                    if it > 0:
                        s_sb = tin.tile([128, PIX], F32, tag="s")
                        nc.vector.tensor_reduce(
                            s_sb,
                            p[:].rearrange("q (pix m) -> q pix m", pix=PIX),
                            axis=AX.X,
                            op=OP.add,
                        )
                        rs = tin.tile([128, PIX], F32, tag="rs")
                        nc.vector.reciprocal(rs, s_sb)

                    # ---- PV = p * V ; mu = (sum_m PV) * rs ----
                    PV = big.tile([128, 2048], prhs_dt, tag="scr")
                    PV4 = PV[:].rearrange("q (pix m k) -> q pix m k", pix=PIX, m=32)
                    p_bck = _bc(p[:], [[32, PIX], [1, 32], [0, 4]])
                    nc.vector.tensor_tensor(PV4, V4, p_bck, op=OP.mult)
                    mu_ps = ps_mu.tile([128, 128], F32, tag="mu")
                    for ms in range(32):
                        nc.tensor.matmul(
                            mu_ps[:, 0:64],
                            ident,
                            PV4[:, :, ms, :],
                            start=(ms == 0),
                            stop=(ms == 31),
                        )
                    if it > 0:
                        mu_sb = sml.tile([128, 64], F32, tag="mu_sb")
                        nc.vector.tensor_tensor(
                            mu_sb[:].rearrange("q (pix k) -> q pix k", pix=PIX),
                            mu_ps[:, 0:64].rearrange("q (pix k) -> q pix k", pix=PIX),
                            _bc(rs[:], [[1, PIX], [0, 4]]),
                            op=OP.mult,
                        )
                        mu_src = mu_sb[:]
                    else:
                        mu_src = mu_ps[:, 0:64]

                    # ---- VC = V - mu ; VCSQ = VC^2 ----
                    VC = big.tile([128, 2048], F32, tag="vc")
                    VC4 = VC[:].rearrange("q (pix m k) -> q pix m k", pix=PIX, m=32)
                    nc.vector.tensor_tensor(
                        VC4, V4, _bc(mu_src, [[4, PIX], [0, 32], [1, 4]]), op=OP.subtract
                    )
                    VCSQ = sqp.tile([128, 2048], F32, tag="vcsq")
                    VCSQ4 = VCSQ[:].rearrange(
                        "q (pix m k) -> q pix m k", pix=PIX, m=32
                    )
                    nc.scalar.activation(VCSQ, VC, AF.Square)

                    # ---- PS = p * VCSQ ; ssq = (sum_m PS) * rs ----
                    PS = big.tile([128, 2048], prhs_dt, tag="scr")
                    PS4 = PS[:].rearrange("q (pix m k) -> q pix m k", pix=PIX, m=32)
                    nc.gpsimd.tensor_tensor(PS4, VCSQ4, p_bck, op=OP.mult)
                    ssq_ps = ps_sq.tile([128, 64], F32, tag="ssq")
                    for ms in range(32):
                        nc.tensor.matmul(
                            ssq_ps[:, 0:64],
                            ident,
                            PS4[:, :, ms, :],
                            start=(ms == 0),
                            stop=(ms == 31),
                        )
                    if it > 0:
                        ssq_sb = sml.tile([128, 64], F32, tag="ssq_sb")
                        nc.vector.tensor_tensor(
                            ssq_sb[:].rearrange("q (pix k) -> q pix k", pix=PIX),
                            ssq_ps[:, 0:64].rearrange("q (pix k) -> q pix k", pix=PIX),
                            _bc(rs[:], [[1, PIX], [0, 4]]),
                            op=OP.mult,
                        )
                        ssq_src = ssq_sb[:]
                    else:
                        ssq_src = ssq_ps[:, 0:64]

                    if it < 2:
                        # ---- log-pdf pieces ----
                        is_sb = sml.tile([128, 64], F32, tag="is")
                        nc.vector.reciprocal(is_sb, ssq_src)
                        lgs = sml.tile([128, 64], F32, tag="lgs")
                        nc.scalar.activation(
                            lgs, ssq_src, AF.Ln, bias=EPS, scale=TWO_PI
                        )
                        T2 = big.tile([128, 2048], prhs_dt, tag="scr")
                        T24 = T2[:].rearrange(
                            "q (pix m k) -> q pix m k", pix=PIX, m=32
                        )
                        nc.gpsimd.tensor_tensor(
                            T24,
                            VCSQ4,
                            _bc(is_sb[:], [[4, PIX], [0, 32], [1, 4]]),
                            op=OP.mult,
                        )
                        d_ps = ps_d.tile([128, 512], F32, tag="dse")
                        for kc in range(4):
                            nc.tensor.matmul(
                                d_ps,
                                bdiag,
                                T24[:, :, :, kc],
                                start=(kc == 0),
                                stop=(kc == 3),
                            )
                        # Slog = sum_d ln(2*pi*ssq + eps)  (k-reduce then i-reduce)
                        lgk = tin.tile([128, PIX], F32, tag="lgk")
                        nc.vector.tensor_reduce(
                            lgk,
                            lgs[:].rearrange("q (pix k) -> q pix k", pix=PIX),
                            axis=AX.X,
                            op=OP.add,
                        )
                        nc.tensor.matmul(
                            mu_ps[:, 64:80], bdiag, lgk, start=True, stop=True
                        )
                        # Kn = ln(spo + eps) - 0.5 * Slog
                        Kn = tin.tile([128, PIX], F32, tag="kn")
                        if it == 0:
                            nc.vector.tensor_scalar(
                                Kn, mu_ps[:, 64:80], -0.5, LNC32, op0=OP.mult,
                                op1=OP.add,
                            )
                        else:
                            nc.tensor.matmul(
                                mu_ps[:, 96:112], onesm, s_sb, start=True, stop=True
                            )
                            rsn = tin.tile([128, PIX], F32, tag="rsn")
                            nc.vector.reciprocal(rsn, mu_ps[:, 96:112])
                            spo = tin.tile([128, PIX], F32, tag="spo")
                            nc.vector.scalar_tensor_tensor(
                                spo, s_sb, 4.0, rsn, op0=OP.mult, op1=OP.mult
                            )
                            if it == 1:
                                spo_save = spo
                            lnspo = tin.tile([128, PIX], F32, tag="lnspo")
                            nc.scalar.activation(lnspo, spo, AF.Ln, bias=EPS)
                            nc.vector.scalar_tensor_tensor(
                                Kn, mu_ps[:, 64:80], -0.5, lnspo, op0=OP.mult,
                                op1=OP.add,
                            )
                        # x = -0.5 * D + Kn ; softmax over n
                        x = med.tile([128, 512], F32, tag="x")
                        nc.vector.scalar_tensor_tensor(
                            x,
                            d_ps,
                            -0.5,
                            _bc(Kn[:], [[1, PIX], [0, 32]]),
                            op0=OP.mult,
                            op1=OP.add,
                        )
                        xm = med.tile([128, 512], F32, tag="xm")
                        nc.gpsimd.partition_all_reduce(
                            xm, x, channels=128, reduce_op=bass_isa.ReduceOp.max
                        )
                        xs = med.tile([128, 512], F32, tag="xs")
                        nc.vector.tensor_tensor(xs, x, xm, op=OP.subtract)
                        ex = med.tile([128, 512], F32, tag="ex")
                        nc.scalar.activation(ex, xs, AF.Exp)
                        se_ps = ps_d.tile([128, 512], F32, tag="dse")
                        nc.tensor.matmul(se_ps, onesm, ex, start=True, stop=True)
                        lt = med.tile([128, 512], F32, tag="lt")
                        nc.scalar.activation(lt, se_ps, AF.Ln)
                        tt = med.tile([128, 512], F32, tag="tt")
                        nc.scalar.activation(tt, lt, AF.Exp, bias=LN4, scale=-1.0)
                        p1 = med.tile([128, 512], F32, tag="p1")
                        nc.vector.tensor_tensor(p1, ex, tt, op=OP.mult)
                        p = ops.tile([128, 512], F32, tag="p")
                        nc.gpsimd.tensor_tensor(p, p1, cp_t, op=OP.mult)
                    else:
                        # ---- activations + output ----
                        lgs2 = sml.tile([128, 64], F32, tag="lgs")
                        nc.scalar.activation(
                            lgs2, ssq_src, AF.Ln, bias=EPS, scale=TWO_PI
                        )
                        lgk2 = tin.tile([128, PIX], F32, tag="lgk")
                        nc.vector.tensor_reduce(
                            lgk2,
                            lgs2[:].rearrange("q (pix k) -> q pix k", pix=PIX),
                            axis=AX.X,
                            op=OP.add,
                        )
                        nc.tensor.matmul(
                            mu_ps[:, 64:80], bdiag, lgk2, start=True, stop=True
                        )
                        g = tin.tile([128, PIX], F32, tag="g")
                        nc.vector.tensor_scalar(
                            g, mu_ps[:, 64:80], 16.0, None, op0=OP.add
                        )
                        w_ = tin.tile([128, PIX], F32, tag="w_")
                        nc.vector.tensor_tensor(w_, g, spo_save, op=OP.mult)
                        acts = tin.tile([128, PIX], F32, tag="acts")
                        nc.scalar.activation(
                            acts, w_, AF.Sigmoid, bias=bias16, scale=-0.5
                        )
                        nc.sync.dma_start(
                            out=out_d[t, :, 0:512].rearrange(
                                "pix (ni k) -> ni pix k", k=4
                            ),
                            in_=mu_sb[:].rearrange("q (pix k) -> q pix k", pix=PIX),
                        )
                        acts_i0 = bass.AP(
                            tensor=acts[:].tensor,
                            offset=acts[:].offset,
                            ap=[[acts[:].ap[0][0] * 4, 32], [1, PIX]],
                        )
                        nc.sync.dma_start(
                            out=out_d[t, :, 512:544].rearrange("pix n -> n pix"),
                            in_=acts_i0,
                        )
    return nc


def _prep(inputs, W, bias):
    arr = inputs.reshape(16, HW, 17, 32)
    pose = arr[:, :, :16, :].reshape(16, HW, 4, 4, 32)  # [b, hw, j, k, m]
    coeff = arr[:, :, 16, :]  # [b, hw, m]
    c = coeff / coeff.sum(-1, keepdims=True)

    P = np.ascontiguousarray(pose.transpose(1, 4, 2, 0, 3))  # [hw, m, j, b, k]
    rhs = np.zeros((HW, 32, 4, PIX, 32, 4), np.float32)
    for m in range(32):
        rhs[:, m, :, :, m, :] = P[:, m]
    rhs = rhs.reshape(HW, 128, 2048)

    wl = np.ascontiguousarray(
        W.reshape(HW, 32, 32, 4, 4).transpose(0, 1, 4, 2, 3)
    ).reshape(HW, 128, 128)

    cT = c.transpose(1, 0, 2)  # [hw, b, m]
    cp = np.ascontiguousarray(
        np.broadcast_to(cT[:, None], (HW, 128, PIX, 32))
    ).reshape(HW, 128, 512)

    cst = np.zeros((4, 128, 128), np.float32)
    cst[0] = np.eye(128, dtype=np.float32)
    cst[1] = np.kron(np.eye(32, dtype=np.float32), np.ones((4, 4), np.float32))
    cst[2] = 1.0
    cst[3, :, 0] = 16.0 * np.repeat(bias.reshape(32), 4)

    in_maps = []
    for cidx in range(N_CORES):
        o, n = OFFS[cidx], COUNTS[cidx]
        idx = list(range(o, o + n)) + [o] * (SLOTS - n)
        in_maps.append(
            {
                "wl": np.ascontiguousarray(wl[idx]),
                "rhs": np.ascontiguousarray(rhs[idx]),
                "cp": np.ascontiguousarray(cp[idx]),
                "cst": cst,
            }
        )
    return in_maps


def _run(in_maps, trace=False):
    if "nc" not in _CACHED:
        _CACHED["nc"] = build_bass()
    return run_bass_kernel_spmd(
        _CACHED["nc"], in_maps, list(range(N_CORES)), trace=trace
    )


def kernel(inputs, W, bias):
    inputs = np.asarray(inputs, np.float32)
    W = np.asarray(W, np.float32)
    bias = np.asarray(bias, np.float32)
    in_maps = _prep(inputs, W, bias)
    res = _run(in_maps).results
    out = np.zeros((16, HW, 544), np.float32)
    for cidx in range(N_CORES):
        o, n = OFFS[cidx], COUNTS[cidx]
        # res[c]["out"]: [SLOTS, PIX, 544] -> out[b, hw, :]
        out[:, o : o + n, :] = res[cidx]["out"][:n].transpose(1, 0, 2)
    return out.reshape(16, 14, 14, 544)


# revision 12
# speedup vs baseline: 1.2864x; 1.2864x over previous
"""Trainium2 Bass kernel for nn_CapsuleLayer (EM-routing capsule layer).

Strategy
--------
Shard the 196 (h,w) pixel positions across 8 cores (25/25/25/25/24/24/24/24,
padded to 25 slots each).  Each (h,w) "tile" covers the 16 batch elements
(16 "pixels" per tile).

On-chip layout: partition q = (n, i)  [n = out-capsule 0..31, i = pose-row
0..3], free = (pix, m, k) [pix = batch 0..15, m = in-capsule 0..31, k =
pose-col 0..3].  d = (i, k).

votes[q, (pix,m,k)] are produced by one PE matmul per tile:
  lhsT = W[hw] as [(m,j), (n,i)]  (128x128)
  rhs  = block-diag pose [(m,j), (pix,m',k)] (built on host, 128x2048)
  out[(n,i),(pix,m,k)] = sum_j W[m,n,i,j] * pose[pix,m,j,k]

Routing (3 EM iterations) runs with:
  - DVE: elementwise tensor-tensor passes + small ops/reciprocals
  - ACT: squares / exp / ln / sigmoid
  - GPSIMD: two of the big elementwise passes + cross-partition max
  - PE: all reductions over m (32 accumulating identity matmuls), over
    (i,k) (block-diag matmuls), over n (all-ones matmuls)

sum over m uses unnormalized priors p; the 1/sum_m normalization is applied
to the (tiny) reduced outputs instead of the big tensors.
softmax over n: exact max via gpsimd.partition_all_reduce; the 1/sum_n
normalization is folded through ln/exp (values are positive there).
"""

import os
import sys

import numpy as np

sys.path.insert(0, "/opt/trn_rl_repo")

import concourse.bass as bass
import concourse.bacc as bacc
import concourse.tile as tile
from concourse import mybir
from concourse.bass_utils import run_bass_kernel_spmd
import concourse.bass_isa as bass_isa

F32 = mybir.dt.float32
BF16 = mybir.dt.bfloat16
AF = mybir.ActivationFunctionType
OP = mybir.AluOpType
AX = mybir.AxisListType

EPS = 1e-7
TWO_PI = 2.0 * np.pi
LNC32 = float(np.log(1.0 / 32.0 + EPS))
LN4 = float(np.log(4.0))

N_CORES = 8
HW = 196
SLOTS = 25  # padded hw tiles per core
PIX = 16  # batch elements per hw tile
COUNTS = [25, 25, 25, 25, 24, 24, 24, 24]
OFFS = np.cumsum([0] + COUNTS)[:-1]

# dtype of the PE-feeding product tensors (PV/PS/T2).  bf16 makes those
# matmul rhs operands run at 1 cycle/row instead of 4.
PE_RHS_BF16 = False
F32R = mybir.dt.float32r
# diagnostics / perf knobs
GPS_TT = True      # big PS/T2 elementwise mults on GPSIMD (else DVE)
EXACT_MAX = True   # exact softmax max via gpsimd.partition_all_reduce
GPS_PMULT = True   # p = p1 * cp on GPSIMD (else DVE)

_CACHED = {}


def _bc(ap, dims):
    """View `ap` (a [128, ...] SBUF/PSUM AP) with custom free dims."""
    return bass.AP(tensor=ap.tensor, offset=ap.offset, ap=[ap.ap[0]] + dims)


def build_bass():
    nc = bacc.Bacc()
    wl_d = nc.dram_tensor("wl", [SLOTS, 128, 128], F32R, kind="ExternalInput")
    rhs_d = nc.dram_tensor("rhs", [SLOTS, 128, 2048], F32R, kind="ExternalInput")
    cp_d = nc.dram_tensor("cp", [SLOTS, 128, 512], F32, kind="ExternalInput")
    cst_d = nc.dram_tensor("cst", [4, 128, 128], F32, kind="ExternalInput")
    cstr_d = nc.dram_tensor("cstr", [1, 128, 128], F32R, kind="ExternalInput")
    out_d = nc.dram_tensor("out", [SLOTS, PIX, 544], F32, kind="ExternalOutput")

    prhs_dt = BF16 if PE_RHS_BF16 else F32

    with tile.TileContext(nc) as tc:
        with (
            tc.tile_pool(name="consts", bufs=1) as consts,
            tc.tile_pool(name="win", bufs=2) as win,
            tc.tile_pool(name="rin", bufs=2) as rin,
            tc.tile_pool(name="cin", bufs=2) as cin,
            tc.tile_pool(name="vbuf", bufs=2) as vbuf,
            tc.tile_pool(name="big", bufs=3) as big,
            tc.tile_pool(name="sq", bufs=2) as sqp,
            tc.tile_pool(name="med", bufs=3) as med,
            tc.tile_pool(name="sml", bufs=4) as sml,
            tc.tile_pool(name="tin", bufs=6) as tin,
            tc.tile_pool(name="ops", bufs=2) as ops,
            tc.tile_pool(name="ps_v", bufs=1, space="PSUM") as ps_v,
            tc.tile_pool(name="ps_mu", bufs=2, space="PSUM") as ps_mu,
            tc.tile_pool(name="ps_sq", bufs=1, space="PSUM") as ps_sq,
            tc.tile_pool(name="ps_d", bufs=1, space="PSUM") as ps_d,
        ):
            ident = consts.tile([128, 128], F32, tag="ident")
            bdiag = consts.tile([128, 128], F32, tag="bdiag")
            onesm = consts.tile([128, 128], F32, tag="onesm")
            bias16 = consts.tile([128, 1], F32, tag="bias16")
            nc.sync.dma_start(out=ident, in_=cst_d[0])
            nc.sync.dma_start(out=bdiag, in_=cst_d[1])
            nc.sync.dma_start(out=onesm, in_=cst_d[2])
            nc.sync.dma_start(out=bias16, in_=cst_d[3, :, 0:1])
            eps_t = consts.tile([128, 1], F32, tag="eps_t")
            ln4_t = consts.tile([128, 1], F32, tag="ln4_t")
            nc.vector.memset(eps_t, EPS)
            nc.vector.memset(ln4_t, LN4)
            ident_r = ident
            bdiag_r = consts.tile([128, 128], F32R, tag="bdiag_r")
            nc.sync.dma_start(out=bdiag_r, in_=cstr_d[0])

            for t in range(SLOTS):
                wl_t = win.tile([128, 128], F32R, tag="wl")
                rhs_t = rin.tile([128, 2048], F32R, tag="rhs")
                cp_t = cin.tile([128, 512], F32, tag="cp")
                nc.sync.dma_start(out=wl_t, in_=wl_d[t])
                nc.sync.dma_start(out=rhs_t, in_=rhs_d[t])
                nc.sync.dma_start(out=cp_t, in_=cp_d[t])

                votes_ps = ps_v.tile([128, 2048], F32, tag="votes")
                for vj in range(4):
                    nc.tensor.matmul(
                        votes_ps[:, vj * 512 : (vj + 1) * 512],
                        wl_t,
                        rhs_t[:, vj * 512 : (vj + 1) * 512],
                        start=True,
                        stop=True,
                    )
                V = vbuf.tile([128, 2048], F32, tag="V")
                nc.vector.tensor_copy(V[:, 0:1024], votes_ps[:, 0:1024])
                nc.scalar.copy(V[:, 1024:2048], votes_ps[:, 1024:2048])
                V4 = V[:].rearrange("q (pix m k) -> q pix m k", pix=PIX, m=32)

                p = cp_t  # iter-0 priors_in == normalized coeffs
                rs = None  # 1/sum_m p (None => sum is exactly 1)
                spo_save = None
                mu_sb = None

                for it in range(3):
                    if it > 0:
                        s_sb = tin.tile([128, PIX], F32, tag="s")
                        nc.vector.tensor_reduce(
                            s_sb,
                            p[:].rearrange("q (pix m) -> q pix m", pix=PIX),
                            axis=AX.X,
                            op=OP.add,
                        )
                        rs = tin.tile([128, PIX], F32, tag="rs")
                        nc.vector.reciprocal(rs, s_sb)

                    # ---- PV = p * V ; mu = (sum_m PV) * rs ----
                    PV = big.tile([128, 2048], prhs_dt, tag="scr")
                    PV4 = PV[:].rearrange("q (pix m k) -> q pix m k", pix=PIX, m=32)
                    p_bck = _bc(p[:], [[32, PIX], [1, 32], [0, 4]])
                    nc.vector.tensor_tensor(PV4, V4, p_bck, op=OP.mult)
                    mu_ps = ps_mu.tile([128, 128], F32, tag="mu")
                    for ms in range(32):
                        nc.tensor.matmul(
                            mu_ps[:, 0:64],
                            ident_r,
                            PV4[:, :, ms, :],
                            start=(ms == 0),
                            stop=(ms == 31),
                        )
                    if it > 0:
                        mu_sb = sml.tile([128, 64], F32, tag="mu_sb")
                        nc.vector.tensor_tensor(
                            mu_sb[:].rearrange("q (pix k) -> q pix k", pix=PIX),
                            mu_ps[:, 0:64].rearrange("q (pix k) -> q pix k", pix=PIX),
                            _bc(rs[:], [[1, PIX], [0, 4]]),
                            op=OP.mult,
                        )
                        mu_src = mu_sb[:]
                    else:
                        mu_src = mu_ps[:, 0:64]

                    # ---- VC = V - mu ; VCSQ = VC^2 ----
                    VC = big.tile([128, 2048], F32, tag="vc")
                    VC4 = VC[:].rearrange("q (pix m k) -> q pix m k", pix=PIX, m=32)
                    nc.vector.tensor_tensor(
                        VC4, V4, _bc(mu_src, [[4, PIX], [0, 32], [1, 4]]), op=OP.subtract
                    )
                    VCSQ = sqp.tile([128, 2048], F32, tag="vcsq")
                    VCSQ4 = VCSQ[:].rearrange(
                        "q (pix m k) -> q pix m k", pix=PIX, m=32
                    )
                    nc.scalar.activation(VCSQ, VC, AF.Square)

                    # ---- PS = p * VCSQ ; ssq = (sum_m PS) * rs ----
                    PS = big.tile([128, 2048], prhs_dt, tag="scr")
                    PS4 = PS[:].rearrange("q (pix m k) -> q pix m k", pix=PIX, m=32)
                    nc.gpsimd.tensor_tensor(PS4, VCSQ4, p_bck, op=OP.mult)
                    ssq_ps = ps_sq.tile([128, 64], F32, tag="ssq")
                    for ms in range(32):
                        nc.tensor.matmul(
                            ssq_ps[:, 0:64],
                            ident_r,
                            PS4[:, :, ms, :],
                            start=(ms == 0),
                            stop=(ms == 31),
                        )
                    if it > 0:
                        ssq_sb = sml.tile([128, 64], F32, tag="ssq_sb")
                        nc.vector.tensor_tensor(
                            ssq_sb[:].rearrange("q (pix k) -> q pix k", pix=PIX),
                            ssq_ps[:, 0:64].rearrange("q (pix k) -> q pix k", pix=PIX),
                            _bc(rs[:], [[1, PIX], [0, 4]]),
                            op=OP.mult,
                        )
                        ssq_src = ssq_sb[:]
                    else:
                        ssq_src = ssq_ps[:, 0:64]

                    if it < 2:
                        # ---- log-pdf pieces ----
                        is_sb = sml.tile([128, 64], F32, tag="is")
                        nc.vector.reciprocal(is_sb, ssq_src)
                        lgs = sml.tile([128, 64], F32, tag="lgs")
                        nc.scalar.activation(
                            lgs, ssq_src, AF.Ln, bias=eps_t[:], scale=TWO_PI
                        )
                        # T2 stored as (pix, k, m) so the per-k matmul rhs
                        # slices have a contiguous inner dim (ISA requirement)
                        T2 = big.tile([128, 2048], F32R, tag="scr")
                        T2w = _bc(T2[:], [[128, PIX], [1, 32], [32, 4]])
                        eng_t2 = nc.gpsimd if GPS_TT else nc.vector
                        eng_t2.tensor_tensor(
                            T2w,
                            VCSQ4,
                            _bc(is_sb[:], [[4, PIX], [0, 32], [1, 4]]),
                            op=OP.mult,
                        )
                        T2k = T2[:].rearrange(
                            "q (pix k m) -> q pix k m", pix=PIX, k=4
                        )
                        d_ps = ps_d.tile([128, 512], F32, tag="dse")
                        for kc in range(4):
                            nc.tensor.matmul(
                                d_ps,
                                bdiag_r,
                                T2k[:, :, kc, :],
                                start=(kc == 0),
                                stop=(kc == 3),
                            )
                        # Slog = sum_d ln(2*pi*ssq + eps)  (k-reduce then i-reduce)
                        lgk = tin.tile([128, PIX], F32, tag="lgk")
                        nc.vector.tensor_reduce(
                            lgk,
                            lgs[:].rearrange("q (pix k) -> q pix k", pix=PIX),
                            axis=AX.X,
                            op=OP.add,
                        )
                        nc.tensor.matmul(
                            mu_ps[:, 64:80], bdiag, lgk, start=True, stop=True
                        )
                        # Kn = ln(spo + eps) - 0.5 * Slog
                        Kn = tin.tile([128, PIX], F32, tag="kn")
                        if it == 0:
                            nc.vector.tensor_scalar(
                                Kn, mu_ps[:, 64:80], -0.5, LNC32, op0=OP.mult,
                                op1=OP.add,
                            )
                        else:
                            nc.tensor.matmul(
                                mu_ps[:, 96:112], onesm, s_sb, start=True, stop=True
                            )
                            rsn = tin.tile([128, PIX], F32, tag="rsn")
                            nc.vector.reciprocal(rsn, mu_ps[:, 96:112])
                            spo = tin.tile([128, PIX], F32, tag="spo")
                            nc.vector.scalar_tensor_tensor(
                                spo, s_sb, 4.0, rsn, op0=OP.mult, op1=OP.mult
                            )
                            if it == 1:
                                spo_save = spo
                            lnspo = tin.tile([128, PIX], F32, tag="lnspo")
                            nc.scalar.activation(lnspo, spo, AF.Ln, bias=eps_t[:])
                            nc.vector.scalar_tensor_tensor(
                                Kn, mu_ps[:, 64:80], -0.5, lnspo, op0=OP.mult,
                                op1=OP.add,
                            )
                        # x = -0.5 * D + Kn ; softmax over n
                        x = med.tile([128, 512], F32, tag="x")
                        nc.vector.scalar_tensor_tensor(
                            x,
                            d_ps,
                            -0.5,
                            _bc(Kn[:], [[1, PIX], [0, 32]]),
                            op0=OP.mult,
                            op1=OP.add,
                        )
                        if EXACT_MAX:
                            xm = med.tile([128, 512], F32, tag="xm")
                            nc.gpsimd.partition_all_reduce(
                                xm, x, channels=128, reduce_op=bass_isa.ReduceOp.max
                            )
                            xs = med.tile([128, 512], F32, tag="xs")
                            nc.vector.tensor_tensor(xs, x, xm, op=OP.subtract)
                        else:
                            xs = x
                        ex = med.tile([128, 512], F32, tag="ex")
                        nc.scalar.activation(ex, xs, AF.Exp)
                        se_ps = ps_d.tile([128, 512], F32, tag="dse")
                        nc.tensor.matmul(se_ps, onesm, ex, start=True, stop=True)
                        lt = med.tile([128, 512], F32, tag="lt")
                        nc.scalar.activation(lt, se_ps, AF.Ln)
                        tt = med.tile([128, 512], F32, tag="tt")
                        nc.scalar.activation(tt, lt, AF.Exp, bias=ln4_t[:], scale=-1.0)
                        p1 = med.tile([128, 512], F32, tag="p1")
                        nc.vector.tensor_tensor(p1, ex, tt, op=OP.mult)
                        p = ops.tile([128, 512], F32, tag="p")
                        (nc.gpsimd if GPS_PMULT else nc.vector).tensor_tensor(
                            p, p1, cp_t, op=OP.mult
                        )
                    else:
                        # ---- activations + output ----
                        lgs2 = sml.tile([128, 64], F32, tag="lgs")
                        nc.scalar.activation(
                            lgs2, ssq_src, AF.Ln, bias=eps_t[:], scale=TWO_PI
                        )
                        lgk2 = tin.tile([128, PIX], F32, tag="lgk")
                        nc.vector.tensor_reduce(
                            lgk2,
                            lgs2[:].rearrange("q (pix k) -> q pix k", pix=PIX),
                            axis=AX.X,
                            op=OP.add,
                        )
                        nc.tensor.matmul(
                            mu_ps[:, 64:80], bdiag, lgk2, start=True, stop=True
                        )
                        g = tin.tile([128, PIX], F32, tag="g")
                        nc.vector.tensor_scalar(
                            g, mu_ps[:, 64:80], 16.0, None, op0=OP.add
                        )
                        w_ = tin.tile([128, PIX], F32, tag="w_")
                        nc.vector.tensor_tensor(w_, g, spo_save, op=OP.mult)
                        acts = tin.tile([128, PIX], F32, tag="acts")
                        nc.scalar.activation(
                            acts, w_, AF.Sigmoid, bias=bias16, scale=-0.5
                        )
                        nc.sync.dma_start(
                            out=out_d[t, :, 0:512].rearrange(
                                "pix (ni k) -> ni pix k", k=4
                            ),
                            in_=mu_sb[:].rearrange("q (pix k) -> q pix k", pix=PIX),
                        )
                        acts_i0 = bass.AP(
                            tensor=acts[:].tensor,
                            offset=acts[:].offset,
                            ap=[[acts[:].ap[0][0] * 4, 32], [1, PIX]],
                        )
                        nc.sync.dma_start(
                            out=out_d[t, :, 512:544].rearrange("pix n -> n pix"),
                            in_=acts_i0,
                        )
    nc.compile()
    return nc


def _prep(inputs, W, bias):
    arr = inputs.reshape(16, HW, 17, 32)
    pose = arr[:, :, :16, :].reshape(16, HW, 4, 4, 32)  # [b, hw, j, k, m]
    coeff = arr[:, :, 16, :]  # [b, hw, m]
    c = coeff / coeff.sum(-1, keepdims=True)

    P = np.ascontiguousarray(pose.transpose(1, 4, 2, 0, 3))  # [hw, m, j, b, k]
    rhs = np.zeros((HW, 32, 4, PIX, 32, 4), np.float32)
    for m in range(32):
        rhs[:, m, :, :, m, :] = P[:, m]
    rhs = rhs.reshape(HW, 128, 2048)

    wl = np.ascontiguousarray(
        W.reshape(HW, 32, 32, 4, 4).transpose(0, 1, 4, 2, 3)
    ).reshape(HW, 128, 128)

    cT = c.transpose(1, 0, 2)  # [hw, b, m]
    cp = np.ascontiguousarray(
        np.broadcast_to(cT[:, None], (HW, 128, PIX, 32))
    ).reshape(HW, 128, 512)

    cst = np.zeros((4, 128, 128), np.float32)
    cst[0] = np.eye(128, dtype=np.float32)
    cst[1] = np.kron(np.eye(32, dtype=np.float32), np.ones((4, 4), np.float32))
    cst[2] = 1.0
    cst[3, :, 0] = 16.0 * np.repeat(bias.reshape(32), 4)

    cstr = cst[1:2].copy()

    in_maps = []
    for cidx in range(N_CORES):
        o, n = OFFS[cidx], COUNTS[cidx]
        idx = list(range(o, o + n)) + [o] * (SLOTS - n)
        in_maps.append(
            {
                "wl": np.ascontiguousarray(wl[idx]),
                "rhs": np.ascontiguousarray(rhs[idx]),
                "cp": np.ascontiguousarray(cp[idx]),
                "cst": cst,
                "cstr": cstr,
            }
        )
    return in_maps


def _run(in_maps, trace=False):
    if "nc" not in _CACHED:
        _CACHED["nc"] = build_bass()
    return run_bass_kernel_spmd(
        _CACHED["nc"], in_maps, list(range(N_CORES)), trace=trace
    )


def kernel(inputs, W, bias):
    inputs = np.asarray(inputs, np.float32)
    W = np.asarray(W, np.float32)
    bias = np.asarray(bias, np.float32)
    in_maps = _prep(inputs, W, bias)
    res = _run(in_maps).results
    out = np.zeros((16, HW, 544), np.float32)
    for cidx in range(N_CORES):
        o, n = OFFS[cidx], COUNTS[cidx]
        # res[c]["out"]: [SLOTS, PIX, 544] -> out[b, hw, :]
        out[:, o : o + n, :] = res[cidx]["out"][:n].transpose(1, 0, 2)
    return out.reshape(16, 14, 14, 544)
